# revision 18
# baseline (speedup 1.0000x reference)
"""Trainium2 Bass kernel for sliding-window ridge/pooling op.

Reference computation (per [B,C,H,W]=[16,1,512,512] f32 input):
    padded = pad W axis right with 16 cols of -1000
    compare[w] = max_{r=1..16}( padded[w+r] - r/10 )
    image = 1 - clip(compare - x, 0, 1)

Algorithm: biased doubling. Define u_k[w] = max_{r=0..k-1}(x[w+r] - r/10).
  u_1 = x
  u_{2k}[w] = max(u_k[w], u_k[w+k] - k/10)      <- one scalar_tensor_tensor op
  compare[w] = u_16[w+1] - 0.1
So 4 STT steps + 1 final STT (d = (u16[w+1]-0.1) - x) + clip + quantize.

The on-chip kernel runs in ~10us; per-call cost is dominated by the axon
tunnel (~43MB/s combined capacity shared by both directions) plus
dispatch latency. So the whole design minimizes wire bytes and overlaps
everything:
  * input is quantized to 8-bit fixed point PER ROW on host (each
    512-pixel row gets its own f32 scale/offset from its exact min/max,
    making 8 bits as accurate as ~9.5 global bits and wrap-proof for any
    input); each 520B wire row = 512 u8 codes + 8 bytes of f32 affine
    (bitcast on device), decoded by one per-partition-scalar affine op,
  * output is quantized to 6 bits and packed 4 values -> 3 bytes on
    device (3MB down instead of 16MB), image = q/63 decoded on host,
  * the jitted PJRT executable is built ONCE and cached (the stock
    run_bass_kernel_spmd path re-traces and re-lowers on every call),
  * donated output buffers are the previous call's device-resident
    output arrays (no zeros upload per call),
  * the batch is cut into CHUNKS slices along H (window is along W, so
    no halo) and dispatched asynchronously with copy_to_host_async, so
    chunk i's download and host decode overlap chunk i+1's pack+upload.

Sharding: data-parallel over batch, 2 images per core on 8 cores.

Error budget (rel 2-norm, gate 2e-2): measured ~1.15e-2 on the seed-0
input (per-row 8-bit input quant + f16 compute + 6-bit output quant;
~70% of output pixels are saturated at exactly 0 or 1 and carry no
quant noise).

Measured end-to-end: ~190ms/call (best-of-12), vs 838ms for the staged
baseline (f32 wire, per-call retrace, serial transfers).
"""

import numpy as np

try:
    from concourse import bacc, bass, bass2jax, mybir
    from concourse.tile import TileContext
except ImportError:  # fallback if site packages not on path
    import sys

    sys.path.insert(0, "/opt/trn_rl_repo")
    from concourse import bacc, bass, bass2jax, mybir
    from concourse.tile import TileContext

N_CORES = 8
B, C, H, W = 16, 1, 512, 512
PB = B // N_CORES            # batches per core = 2
P = 128                      # SBUF partitions
PAD_VAL = -1000.0
BUFW = W + 16                # 528: 512 data + 16 window pad (exact minimum)
ROWB = W + 8                 # 520 wire bytes/row: 512 u8 codes + 2 f32 affine
OW = (W * 3) // 4            # 384 output bytes/row: 6-bit packed, 3 planes
Q = W // 4                   # 128 values per phase/plane

CHUNKS = 16                  # pipeline chunks along H
HC = H // CHUNKS             # rows per chunk
ROWS = PB * C * HC           # rows per core per chunk
PP = min(P, ROWS)            # partitions used per tile
SEGS = max(ROWS // P, 1)     # SBUF segments per core per chunk

_state = {}


def _build_nc():
    f16 = mybir.dt.float16
    f32 = mybir.dt.float32
    u8d = mybir.dt.uint8
    A = mybir.AluOpType
    sub, mx, mn, mult, add = A.subtract, A.max, A.min, A.mult, A.add
    band, shr, shl, bor = (A.bitwise_and, A.logical_shift_right,
                           A.logical_shift_left, A.bitwise_or)

    nc = bacc.Bacc("TRN2", target_bir_lowering=False, debug=False,
                   num_devices=N_CORES)
    x_dram = nc.dram_tensor("packed", [PB, C, HC, ROWB], u8d,
                            kind="ExternalInput").ap()
    y_dram = nc.dram_tensor("image", [PB, C, HC, OW], u8d,
                            kind="ExternalOutput").ap()
    xf = x_dram.flatten_outer_dims().rearrange("(s p) w -> p s w", p=PP)
    yf = y_dram.flatten_outer_dims().rearrange("(s p) w -> p s w", p=PP)

    CW = BUFW
    with TileContext(nc) as tc:
        with tc.tile_pool(name="io", bufs=SEGS) as iop, \
             tc.tile_pool(name="mid", bufs=SEGS) as midp:
            for s in range(SEGS):
                raw = iop.tile([PP, ROWB], u8d, tag="raw")
                nc.sync.dma_start(out=raw[:], in_=xf[:, s, :])
                # last 8 bytes of each wire row are (row_step, row_min) f32
                aff = raw[:, W:ROWB].bitcast(f32)
                # decode: x = q*row_step + row_min, one tensor_scalar with
                # per-partition (per-row) f32 scalars.
                x = midp.tile([PP, CW], f16, tag="x")
                nc.vector.memset(x[:, W:CW], PAD_VAL)
                nc.vector.tensor_scalar(out=x[:, 0:W], in0=raw[:, 0:W],
                                        scalar1=aff[:, 0:1],
                                        scalar2=aff[:, 1:2],
                                        op0=mult, op1=add)

                u2 = midp.tile([PP, CW], f16, tag="u2")
                nc.vector.scalar_tensor_tensor(
                    out=u2[:, 0:CW - 1], in0=x[:, 1:CW], scalar=0.1,
                    in1=x[:, 0:CW - 1], op0=sub, op1=mx)
                u4 = midp.tile([PP, CW], f16, tag="u4")
                nc.vector.scalar_tensor_tensor(
                    out=u4[:, 0:CW - 3], in0=u2[:, 2:CW - 1], scalar=0.2,
                    in1=u2[:, 0:CW - 3], op0=sub, op1=mx)
                u8t = midp.tile([PP, CW], f16, tag="u8")
                nc.vector.scalar_tensor_tensor(
                    out=u8t[:, 0:CW - 7], in0=u4[:, 4:CW - 3], scalar=0.4,
                    in1=u4[:, 0:CW - 7], op0=sub, op1=mx)
                u16 = midp.tile([PP, CW], f16, tag="u16")
                nc.vector.scalar_tensor_tensor(
                    out=u16[:, 0:CW - 15], in0=u8t[:, 8:CW - 7], scalar=0.8,
                    in1=u8t[:, 0:CW - 15], op0=sub, op1=mx)

                d = midp.tile([PP, CW], f16, tag="d")
                nc.vector.scalar_tensor_tensor(
                    out=d[:, 0:W], in0=u16[:, 1:W + 1], scalar=0.1,
                    in1=x[:, 0:W], op0=sub, op1=sub)
                # t = clip(d, 0, 1); q6 = 63 - 63*t  (image = q6/63)
                # the DVE f16->u8 store rounds to nearest on HW (CoreSim
                # truncates), so no rounding bias is added here.
                t = midp.tile([PP, CW], f16, tag="t")
                nc.vector.tensor_scalar(
                    out=t[:, 0:W], in0=d[:, 0:W],
                    scalar1=0.0, scalar2=1.0, op0=mx, op1=mn)
                q6 = midp.tile([PP, W], u8d, tag="q6")
                nc.vector.tensor_scalar(
                    out=q6[:], in0=t[:, 0:W],
                    scalar1=-63.0, scalar2=63.0, op0=mult, op1=add)
                # pack 4x 6-bit -> 3 byte planes per row:
                #   b0 = q0 | (q1&3)<<6;  b1 = q1>>2 | (q2&15)<<4
                #   b2 = q2>>4 | q3<<2   (q3<<2 <= 252, no overflow)
                # (the walrus verifier rejects bitvec scalar_tensor_tensor
                # with immediates, so shifts go through tensor_scalar and
                # the combines through tensor_tensor)
                q64 = q6[:].rearrange("p (w four) -> p four w", four=4)
                zt = midp.tile([PP, 5 * Q], u8d, tag="zt")
                out = iop.tile([PP, OW], u8d, tag="out")
                z1, z2, z3 = zt[:, 0:Q], zt[:, Q:2 * Q], zt[:, 2 * Q:3 * Q]
                y1, y2 = zt[:, 3 * Q:4 * Q], zt[:, 4 * Q:5 * Q]
                nc.vector.tensor_scalar(out=z1, in0=q64[:, 1, :],
                                        scalar1=3, scalar2=6,
                                        op0=band, op1=shl)
                nc.vector.tensor_tensor(out=out[:, 0:Q], in0=q64[:, 0, :],
                                        in1=z1, op=bor)
                nc.vector.tensor_scalar(out=z2, in0=q64[:, 2, :],
                                        scalar1=15, scalar2=4,
                                        op0=band, op1=shl)
                nc.vector.tensor_scalar(out=y1, in0=q64[:, 1, :],
                                        scalar1=2, scalar2=None, op0=shr)
                nc.vector.tensor_tensor(out=out[:, Q:2 * Q], in0=y1,
                                        in1=z2, op=bor)
                nc.vector.tensor_scalar(out=z3, in0=q64[:, 3, :],
                                        scalar1=2, scalar2=None, op0=shl)
                nc.vector.tensor_scalar(out=y2, in0=q64[:, 2, :],
                                        scalar1=4, scalar2=None, op0=shr)
                nc.vector.tensor_tensor(out=out[:, 2 * Q:3 * Q], in0=y2,
                                        in1=z3, op=bor)
                nc.sync.dma_start(out=yf[:, s, :], in_=out[:])
    nc.compile()
    return nc


def _ensure_fast():
    """Build the Bass module and a cached jitted PJRT executable once.

    Mirrors the multi-core branch of bass2jax.run_bass_via_pjrt, but keeps
    the jax.jit wrapper (and with it the traced/lowered/compiled NEFF
    executable) alive across calls instead of rebuilding it per call.
    """
    if "fn" in _state:
        return
    import jax
    from jax.experimental.shard_map import shard_map
    from jax.sharding import Mesh, PartitionSpec

    bass2jax.install_neuronx_cc_hook()
    nc = _build_nc()

    partition_name = (nc.partition_id_tensor.name
                      if nc.partition_id_tensor else None)
    in_names = []
    out_names = []
    out_avals = []
    for alloc in nc.m.functions[0].allocations:
        if not isinstance(alloc, mybir.MemoryLocationSet):
            continue
        name = alloc.memorylocations[0].name
        if alloc.kind == "ExternalInput":
            if name != partition_name:
                in_names.append(name)
        elif alloc.kind == "ExternalOutput":
            shape = tuple(alloc.tensor_shape)
            dtype = mybir.dt.np(alloc.dtype)
            out_names.append(name)
            out_avals.append(jax.core.ShapedArray(shape, dtype))
    n_params = len(in_names)
    in_names = in_names + out_names  # donated output buffers come in as params
    if partition_name is not None:
        in_names.append(partition_name)

    def _body(*args):
        operands = list(args)
        if partition_name is not None:
            operands.append(bass2jax.partition_id_tensor())
        outs = bass2jax._bass_exec_p.bind(
            *operands,
            out_avals=tuple(out_avals),
            in_names=tuple(in_names),
            out_names=tuple(out_names),
            lowering_input_output_aliases=(),
            sim_require_finite=True,
            sim_require_nnan=True,
            nc=nc,
        )
        return tuple(outs)

    devices = jax.devices()[:N_CORES]
    mesh = Mesh(np.asarray(devices), ("core",))
    n_outs = len(out_names)
    fn = jax.jit(
        shard_map(_body, mesh=mesh,
                  in_specs=(PartitionSpec("core"),) * (n_params + n_outs),
                  out_specs=(PartitionSpec("core"),) * n_outs,
                  check_rep=False),
        donate_argnums=tuple(range(n_params, n_params + n_outs)),
        keep_unused=True,
    )
    _state["nc"] = nc
    _state["fn"] = fn
    # First call donates host zeros; afterwards we donate the previous
    # call's device-resident output arrays (already fetched to host).
    _state["donate"] = [np.zeros((B, C, HC, OW), np.uint8)
                        for _ in range(CHUNKS)]
    # Host-side scratch, reused across chunks/calls. packed is per-chunk
    # (its async upload outlives the pack loop iteration).
    _state["packed"] = [np.empty((B, C, HC, ROWB), np.uint8)
                        for _ in range(CHUNKS)]
    _state["fbuf"] = np.empty((B, C, HC, W), np.float32)
    _state["qful"] = np.empty((B, C, HC, W), np.uint8)
    _state["lut"] = (np.arange(64) / 63.0).astype(np.float32)
    from concurrent.futures import ThreadPoolExecutor
    _state["pool"] = ThreadPoolExecutor(1)


def _pack_chunk(xc, packed):
    """Quantize one [B,C,HC,W] f32 chunk to per-row 8-bit codes.

    q = round((x - rowmin) * 255/(rowmax-rowmin)); decode on device is
    x = q*rowstep + rowmin. Exact row bounds make any input wrap-proof.
    The two f32 affines live in the last 8 bytes of each 520B wire row.
    """
    fbuf = _state["fbuf"]
    mn = xc.min(axis=-1, keepdims=True)
    mx = xc.max(axis=-1, keepdims=True)
    diff = np.maximum(mx - mn, np.float32(1e-9))
    senc = np.float32(255.0) / diff
    np.multiply(xc, senc, out=fbuf)
    fbuf -= senc * mn - np.float32(0.5)        # +0.5: round via truncation
    np.copyto(packed[..., :W], fbuf, casting="unsafe")  # f32->u8 truncation
    affv = packed[..., W:].view(np.float32)
    affv[..., 0] = (diff * np.float32(1.0 / 255.0))[..., 0]
    affv[..., 1] = mn[..., 0]


def _decode_chunk(res_u8, view):
    """Unpack one [B,C,HC,OW] 6-bit plane chunk into f32 `view`."""
    qful, lut = _state["qful"], _state["lut"]
    b0 = res_u8[..., 0:Q]
    b1 = res_u8[..., Q:2 * Q]
    b2 = res_u8[..., 2 * Q:3 * Q]
    qful[..., 0::4] = b0 & 63
    qful[..., 1::4] = (b0 >> 6) | ((b1 & 15) << 2)
    qful[..., 2::4] = (b1 >> 4) | ((b2 & 3) << 4)
    qful[..., 3::4] = b2 >> 2
    np.take(lut, qful, out=view)


def _run_fast(heightfield: np.ndarray) -> np.ndarray:
    _ensure_fast()
    hf = np.asarray(heightfield, dtype=np.float32)
    assert hf.shape == (B, C, H, W), hf.shape
    fn = _state["fn"]
    donate = _state["donate"]
    pool = _state["pool"]
    result = np.empty((B, C, H, W), np.float32)

    def _fetch(i, o):
        res_u8 = np.asarray(o)                # blocks on chunk i only
        donate[i] = o                         # device buffer, donated next call
        _decode_chunk(res_u8, result[:, :, i * HC:(i + 1) * HC, :])

    # Dispatch all chunks asynchronously: uploads stream in order while
    # downloads of finished chunks flow back concurrently (duplex tunnel).
    # A single worker thread fetches + decodes finished chunks in order,
    # overlapping the remaining dispatches (numpy/jax release the GIL).
    futs = []
    for i in range(CHUNKS):
        packed = _state["packed"][i]
        _pack_chunk(hf[:, :, i * HC:(i + 1) * HC, :], packed)
        o = fn(packed, donate[i])[0]
        o.copy_to_host_async()
        futs.append(pool.submit(_fetch, i, o))
    for f in futs:
        f.result()
    return result


def kernel(heightfield: np.ndarray) -> np.ndarray:
    return _run_fast(heightfield)


# revision 19
# speedup vs baseline: 1.0450x; 1.0450x over previous
"""Trainium2 Bass kernel for sliding-window ridge/pooling op.

Reference computation (per [B,C,H,W]=[16,1,512,512] f32 input):
    padded = pad W axis right with 16 cols of -1000
    compare[w] = max_{r=1..16}( padded[w+r] - r/10 )
    image = 1 - clip(compare - x, 0, 1)

Algorithm: biased doubling. Define u_k[w] = max_{r=0..k-1}(x[w+r] - r/10).
  u_1 = x
  u_{2k}[w] = max(u_k[w], u_k[w+k] - k/10)      <- one scalar_tensor_tensor op
  compare[w] = u_16[w+1] - 0.1
So 4 STT steps + 1 final STT (d = (u16[w+1]-0.1) - x) + clip + quantize.

The on-chip kernel runs in ~10us; per-call cost is dominated by the axon
tunnel (~43MB/s combined capacity shared by both directions) plus
dispatch latency. So the whole design minimizes wire bytes and overlaps
everything:
  * input is quantized to 8-bit fixed point PER ROW on host (each
    512-pixel row gets its own f32 scale/offset from its exact min/max,
    making 8 bits as accurate as ~9.5 global bits and wrap-proof for any
    input); each 520B wire row = 512 u8 codes + 8 bytes of f32 affine
    (bitcast on device), decoded by one per-partition-scalar affine op,
  * output is quantized to 6 bits and packed 4 values -> 3 bytes on
    device (3MB down instead of 16MB), image = q/63 decoded on host,
  * the jitted PJRT executable is built ONCE and cached (the stock
    run_bass_kernel_spmd path re-traces and re-lowers on every call),
  * donated output buffers are the previous call's device-resident
    output arrays (no zeros upload per call),
  * the batch is cut into CHUNKS slices along H (window is along W, so
    no halo) and dispatched asynchronously with copy_to_host_async, so
    chunk i's download and host decode overlap chunk i+1's pack+upload.

Sharding: data-parallel over batch, 2 images per core on 8 cores.

Error budget (rel 2-norm, gate 2e-2): measured ~1.15e-2 on the seed-0
input (per-row 8-bit input quant + f16 compute + 6-bit output quant;
~70% of output pixels are saturated at exactly 0 or 1 and carry no
quant noise).

Measured end-to-end: ~190ms/call (best-of-12), vs 838ms for the staged
baseline (f32 wire, per-call retrace, serial transfers).
"""

import numpy as np

try:
    from concourse import bacc, bass, bass2jax, mybir
    from concourse.tile import TileContext
except ImportError:  # fallback if site packages not on path
    import sys

    sys.path.insert(0, "/opt/trn_rl_repo")
    from concourse import bacc, bass, bass2jax, mybir
    from concourse.tile import TileContext

N_CORES = 8
B, C, H, W = 16, 1, 512, 512
PB = B // N_CORES            # batches per core = 2
P = 128                      # SBUF partitions
PAD_VAL = -1000.0
BUFW = W + 16                # 528: 512 data + 16 window pad (exact minimum)
ROWB = W + 8                 # 520 wire bytes/row: 512 u8 codes + 2 f32 affine
OW = (W * 3) // 4            # 384 output bytes/row: 6-bit packed, 3 planes
Q = W // 4                   # 128 values per phase/plane

CHUNKS = 8                   # pipeline chunks along H
HC = H // CHUNKS             # rows per chunk
ROWS = PB * C * HC           # rows per core per chunk
PP = min(P, ROWS)            # partitions used per tile
SEGS = max(ROWS // P, 1)     # SBUF segments per core per chunk

_state = {}


def _build_nc():
    f16 = mybir.dt.float16
    f32 = mybir.dt.float32
    u8d = mybir.dt.uint8
    A = mybir.AluOpType
    sub, mx, mn, mult, add = A.subtract, A.max, A.min, A.mult, A.add
    band, shr, shl, bor = (A.bitwise_and, A.logical_shift_right,
                           A.logical_shift_left, A.bitwise_or)

    nc = bacc.Bacc("TRN2", target_bir_lowering=False, debug=False,
                   num_devices=N_CORES)
    x_dram = nc.dram_tensor("packed", [PB, C, HC, ROWB], u8d,
                            kind="ExternalInput").ap()
    y_dram = nc.dram_tensor("image", [PB, C, HC, OW], u8d,
                            kind="ExternalOutput").ap()
    xf = x_dram.flatten_outer_dims().rearrange("(s p) w -> p s w", p=PP)
    yf = y_dram.flatten_outer_dims().rearrange("(s p) w -> p s w", p=PP)

    CW = BUFW
    with TileContext(nc) as tc:
        with tc.tile_pool(name="io", bufs=SEGS) as iop, \
             tc.tile_pool(name="mid", bufs=SEGS) as midp:
            for s in range(SEGS):
                raw = iop.tile([PP, ROWB], u8d, tag="raw")
                nc.sync.dma_start(out=raw[:], in_=xf[:, s, :])
                # last 8 bytes of each wire row are (row_step, row_min) f32
                aff = raw[:, W:ROWB].bitcast(f32)
                # decode: x = q*row_step + row_min, one tensor_scalar with
                # per-partition (per-row) f32 scalars.
                x = midp.tile([PP, CW], f16, tag="x")
                nc.vector.memset(x[:, W:CW], PAD_VAL)
                nc.vector.tensor_scalar(out=x[:, 0:W], in0=raw[:, 0:W],
                                        scalar1=aff[:, 0:1],
                                        scalar2=aff[:, 1:2],
                                        op0=mult, op1=add)

                u2 = midp.tile([PP, CW], f16, tag="u2")
                nc.vector.scalar_tensor_tensor(
                    out=u2[:, 0:CW - 1], in0=x[:, 1:CW], scalar=0.1,
                    in1=x[:, 0:CW - 1], op0=sub, op1=mx)
                u4 = midp.tile([PP, CW], f16, tag="u4")
                nc.vector.scalar_tensor_tensor(
                    out=u4[:, 0:CW - 3], in0=u2[:, 2:CW - 1], scalar=0.2,
                    in1=u2[:, 0:CW - 3], op0=sub, op1=mx)
                u8t = midp.tile([PP, CW], f16, tag="u8")
                nc.vector.scalar_tensor_tensor(
                    out=u8t[:, 0:CW - 7], in0=u4[:, 4:CW - 3], scalar=0.4,
                    in1=u4[:, 0:CW - 7], op0=sub, op1=mx)
                u16 = midp.tile([PP, CW], f16, tag="u16")
                nc.vector.scalar_tensor_tensor(
                    out=u16[:, 0:CW - 15], in0=u8t[:, 8:CW - 7], scalar=0.8,
                    in1=u8t[:, 0:CW - 15], op0=sub, op1=mx)

                d = midp.tile([PP, CW], f16, tag="d")
                nc.vector.scalar_tensor_tensor(
                    out=d[:, 0:W], in0=u16[:, 1:W + 1], scalar=0.1,
                    in1=x[:, 0:W], op0=sub, op1=sub)
                # t = clip(d, 0, 1); q6 = 63 - 63*t  (image = q6/63)
                # the DVE f16->u8 store rounds to nearest on HW (CoreSim
                # truncates), so no rounding bias is added here.
                t = midp.tile([PP, CW], f16, tag="t")
                nc.vector.tensor_scalar(
                    out=t[:, 0:W], in0=d[:, 0:W],
                    scalar1=0.0, scalar2=1.0, op0=mx, op1=mn)
                q6 = midp.tile([PP, W], u8d, tag="q6")
                nc.vector.tensor_scalar(
                    out=q6[:], in0=t[:, 0:W],
                    scalar1=-63.0, scalar2=63.0, op0=mult, op1=add)
                # pack 4x 6-bit -> 3 byte planes per row:
                #   b0 = q0 | (q1&3)<<6;  b1 = q1>>2 | (q2&15)<<4
                #   b2 = q2>>4 | q3<<2   (q3<<2 <= 252, no overflow)
                # (the walrus verifier rejects bitvec scalar_tensor_tensor
                # with immediates, so shifts go through tensor_scalar and
                # the combines through tensor_tensor)
                q64 = q6[:].rearrange("p (w four) -> p four w", four=4)
                zt = midp.tile([PP, 5 * Q], u8d, tag="zt")
                out = iop.tile([PP, OW], u8d, tag="out")
                z1, z2, z3 = zt[:, 0:Q], zt[:, Q:2 * Q], zt[:, 2 * Q:3 * Q]
                y1, y2 = zt[:, 3 * Q:4 * Q], zt[:, 4 * Q:5 * Q]
                nc.vector.tensor_scalar(out=z1, in0=q64[:, 1, :],
                                        scalar1=3, scalar2=6,
                                        op0=band, op1=shl)
                nc.vector.tensor_tensor(out=out[:, 0:Q], in0=q64[:, 0, :],
                                        in1=z1, op=bor)
                nc.vector.tensor_scalar(out=z2, in0=q64[:, 2, :],
                                        scalar1=15, scalar2=4,
                                        op0=band, op1=shl)
                nc.vector.tensor_scalar(out=y1, in0=q64[:, 1, :],
                                        scalar1=2, scalar2=None, op0=shr)
                nc.vector.tensor_tensor(out=out[:, Q:2 * Q], in0=y1,
                                        in1=z2, op=bor)
                nc.vector.tensor_scalar(out=z3, in0=q64[:, 3, :],
                                        scalar1=2, scalar2=None, op0=shl)
                nc.vector.tensor_scalar(out=y2, in0=q64[:, 2, :],
                                        scalar1=4, scalar2=None, op0=shr)
                nc.vector.tensor_tensor(out=out[:, 2 * Q:3 * Q], in0=y2,
                                        in1=z3, op=bor)
                nc.sync.dma_start(out=yf[:, s, :], in_=out[:])
    nc.compile()
    return nc


def _ensure_fast():
    """Build the Bass module and a cached jitted PJRT executable once.

    Mirrors the multi-core branch of bass2jax.run_bass_via_pjrt, but keeps
    the jax.jit wrapper (and with it the traced/lowered/compiled NEFF
    executable) alive across calls instead of rebuilding it per call.
    """
    if "fn" in _state:
        return
    import jax
    from jax.experimental.shard_map import shard_map
    from jax.sharding import Mesh, PartitionSpec

    bass2jax.install_neuronx_cc_hook()
    nc = _build_nc()

    partition_name = (nc.partition_id_tensor.name
                      if nc.partition_id_tensor else None)
    in_names = []
    out_names = []
    out_avals = []
    for alloc in nc.m.functions[0].allocations:
        if not isinstance(alloc, mybir.MemoryLocationSet):
            continue
        name = alloc.memorylocations[0].name
        if alloc.kind == "ExternalInput":
            if name != partition_name:
                in_names.append(name)
        elif alloc.kind == "ExternalOutput":
            shape = tuple(alloc.tensor_shape)
            dtype = mybir.dt.np(alloc.dtype)
            out_names.append(name)
            out_avals.append(jax.core.ShapedArray(shape, dtype))
    n_params = len(in_names)
    in_names = in_names + out_names  # donated output buffers come in as params
    if partition_name is not None:
        in_names.append(partition_name)

    def _body(*args):
        operands = list(args)
        if partition_name is not None:
            operands.append(bass2jax.partition_id_tensor())
        outs = bass2jax._bass_exec_p.bind(
            *operands,
            out_avals=tuple(out_avals),
            in_names=tuple(in_names),
            out_names=tuple(out_names),
            lowering_input_output_aliases=(),
            sim_require_finite=True,
            sim_require_nnan=True,
            nc=nc,
        )
        return tuple(outs)

    devices = jax.devices()[:N_CORES]
    mesh = Mesh(np.asarray(devices), ("core",))
    n_outs = len(out_names)
    fn = jax.jit(
        shard_map(_body, mesh=mesh,
                  in_specs=(PartitionSpec("core"),) * (n_params + n_outs),
                  out_specs=(PartitionSpec("core"),) * n_outs,
                  check_rep=False),
        donate_argnums=tuple(range(n_params, n_params + n_outs)),
        keep_unused=True,
    )
    _state["nc"] = nc
    _state["fn"] = fn
    # First call donates host zeros; afterwards we donate the previous
    # call's device-resident output arrays (already fetched to host).
    _state["donate"] = [np.zeros((B, C, HC, OW), np.uint8)
                        for _ in range(CHUNKS)]
    # Host-side scratch, reused across chunks/calls. packed is per-chunk
    # (its async upload outlives the pack loop iteration).
    _state["packed"] = [np.empty((B, C, HC, ROWB), np.uint8)
                        for _ in range(CHUNKS)]
    _state["fbuf"] = np.empty((B, C, HC, W), np.float32)
    _state["qful"] = np.empty((B, C, HC, W), np.uint8)
    _state["lut"] = (np.arange(64) / 63.0).astype(np.float32)
    from concurrent.futures import ThreadPoolExecutor
    _state["pool"] = ThreadPoolExecutor(1)


def _pack_chunk(xc, packed):
    """Quantize one [B,C,HC,W] f32 chunk to per-row 8-bit codes.

    q = round((x - rowmin) * 255/(rowmax-rowmin)); decode on device is
    x = q*rowstep + rowmin. Exact row bounds make any input wrap-proof.
    The two f32 affines live in the last 8 bytes of each 520B wire row.
    """
    fbuf = _state["fbuf"]
    mn = xc.min(axis=-1, keepdims=True)
    mx = xc.max(axis=-1, keepdims=True)
    diff = np.maximum(mx - mn, np.float32(1e-9))
    senc = np.float32(255.0) / diff
    np.multiply(xc, senc, out=fbuf)
    fbuf -= senc * mn - np.float32(0.5)        # +0.5: round via truncation
    np.copyto(packed[..., :W], fbuf, casting="unsafe")  # f32->u8 truncation
    affv = packed[..., W:].view(np.float32)
    affv[..., 0] = (diff * np.float32(1.0 / 255.0))[..., 0]
    affv[..., 1] = mn[..., 0]


def _decode_chunk(res_u8, view):
    """Unpack one [B,C,HC,OW] 6-bit plane chunk into f32 `view`."""
    qful, lut = _state["qful"], _state["lut"]
    b0 = res_u8[..., 0:Q]
    b1 = res_u8[..., Q:2 * Q]
    b2 = res_u8[..., 2 * Q:3 * Q]
    qful[..., 0::4] = b0 & 63
    qful[..., 1::4] = (b0 >> 6) | ((b1 & 15) << 2)
    qful[..., 2::4] = (b1 >> 4) | ((b2 & 3) << 4)
    qful[..., 3::4] = b2 >> 2
    np.take(lut, qful, out=view)


def _run_fast(heightfield: np.ndarray) -> np.ndarray:
    _ensure_fast()
    hf = np.asarray(heightfield, dtype=np.float32)
    assert hf.shape == (B, C, H, W), hf.shape
    fn = _state["fn"]
    donate = _state["donate"]
    pool = _state["pool"]
    result = np.empty((B, C, H, W), np.float32)

    def _fetch(i, o):
        res_u8 = np.asarray(o)                # blocks on chunk i only
        donate[i] = o                         # device buffer, donated next call
        _decode_chunk(res_u8, result[:, :, i * HC:(i + 1) * HC, :])

    # Dispatch all chunks asynchronously: uploads stream in order while
    # downloads of finished chunks flow back concurrently (duplex tunnel).
    # A single worker thread fetches + decodes finished chunks in order,
    # overlapping the remaining dispatches (numpy/jax release the GIL).
    futs = []
    for i in range(CHUNKS):
        packed = _state["packed"][i]
        _pack_chunk(hf[:, :, i * HC:(i + 1) * HC, :], packed)
        o = fn(packed, donate[i])[0]
        o.copy_to_host_async()
        futs.append(pool.submit(_fetch, i, o))
    for f in futs:
        f.result()
    return result


def kernel(heightfield: np.ndarray) -> np.ndarray:
    return _run_fast(heightfield)


# revision 20
# speedup vs baseline: 1.0888x; 1.0418x over previous
"""Trainium2 Bass kernel for sliding-window ridge/pooling op.

Reference computation (per [B,C,H,W]=[16,1,512,512] f32 input):
    padded = pad W axis right with 16 cols of -1000
    compare[w] = max_{r=1..16}( padded[w+r] - r/10 )
    image = 1 - clip(compare - x, 0, 1)

Algorithm: biased doubling. Define u_k[w] = max_{r=0..k-1}(x[w+r] - r/10).
  u_1 = x
  u_{2k}[w] = max(u_k[w], u_k[w+k] - k/10)      <- one scalar_tensor_tensor op
  compare[w] = u_16[w+1] - 0.1
So 4 STT steps + 1 final STT (d = (u16[w+1]-0.1) - x) + clip + quantize.

The on-chip kernel runs in ~10us; per-call cost is dominated by the axon
tunnel (~43MB/s combined capacity shared by both directions) plus
dispatch latency. So the whole design minimizes wire bytes and overlaps
everything:
  * input is quantized to 8-bit fixed point PER ROW on host (each
    512-pixel row gets its own f32 scale/offset from its exact min/max,
    making 8 bits as accurate as ~9.5 global bits and wrap-proof for any
    input); each 520B wire row = 512 u8 codes + 8 bytes of f32 affine
    (bitcast on device), decoded by one per-partition-scalar affine op,
  * output is quantized to 6 bits and packed 4 values -> 3 bytes on
    device (3MB down instead of 16MB), image = q/63 decoded on host,
  * the jitted PJRT executable is built ONCE and cached (the stock
    run_bass_kernel_spmd path re-traces and re-lowers on every call),
  * donated output buffers are the previous call's device-resident
    output arrays (no zeros upload per call),
  * the batch is cut into CHUNKS slices along H (window is along W, so
    no halo) and dispatched asynchronously with copy_to_host_async, so
    chunk i's download and host decode overlap chunk i+1's pack+upload.

Sharding: data-parallel over batch, 2 images per core on 8 cores.

Error budget (rel 2-norm, gate 2e-2): measured 1.25e-2 on the seed-0
input (per-row 8-bit input quant + f16 compute + 6-bit output quant;
~70% of output pixels are saturated at exactly 0 or 1 and carry no
quant noise).

Measured end-to-end: ~200-210ms/call (best-of-12), vs 838ms for the
staged baseline (f32 wire, per-call retrace, serial transfers). The
pipeline sits at the tunnel's shared-capacity floor for 7.06MB of wire
traffic (~152ms) plus ~10ms spin-up and ~40ms of tail latency.
"""

import numpy as np

try:
    from concourse import bacc, bass, bass2jax, mybir
    from concourse.tile import TileContext
except ImportError:  # fallback if site packages not on path
    import sys

    sys.path.insert(0, "/opt/trn_rl_repo")
    from concourse import bacc, bass, bass2jax, mybir
    from concourse.tile import TileContext

N_CORES = 8
B, C, H, W = 16, 1, 512, 512
PB = B // N_CORES            # batches per core = 2
P = 128                      # SBUF partitions
PAD_VAL = -1000.0
BUFW = W + 16                # 528: 512 data + 16 window pad (exact minimum)
ROWB = W + 8                 # 520 wire bytes/row: 512 u8 codes + 2 f32 affine
OW = (W * 3) // 4            # 384 output bytes/row: 6-bit packed, 3 planes
Q = W // 4                   # 128 values per phase/plane

CHUNKS = 8                   # pipeline chunks along H
HC = H // CHUNKS             # rows per chunk
ROWS = PB * C * HC           # rows per core per chunk
PP = min(P, ROWS)            # partitions used per tile
SEGS = max(ROWS // P, 1)     # SBUF segments per core per chunk

_state = {}


def _build_nc():
    f16 = mybir.dt.float16
    f32 = mybir.dt.float32
    u8d = mybir.dt.uint8
    A = mybir.AluOpType
    sub, mx, mn, mult, add = A.subtract, A.max, A.min, A.mult, A.add
    band, shr, shl, bor = (A.bitwise_and, A.logical_shift_right,
                           A.logical_shift_left, A.bitwise_or)

    nc = bacc.Bacc("TRN2", target_bir_lowering=False, debug=False,
                   num_devices=N_CORES)
    x_dram = nc.dram_tensor("packed", [PB, C, HC, ROWB], u8d,
                            kind="ExternalInput").ap()
    y_dram = nc.dram_tensor("image", [PB, C, HC, OW], u8d,
                            kind="ExternalOutput").ap()
    xf = x_dram.flatten_outer_dims().rearrange("(s p) w -> p s w", p=PP)
    yf = y_dram.flatten_outer_dims().rearrange("(s p) w -> p s w", p=PP)

    CW = BUFW
    with TileContext(nc) as tc:
        with tc.tile_pool(name="io", bufs=SEGS) as iop, \
             tc.tile_pool(name="mid", bufs=SEGS) as midp:
            for s in range(SEGS):
                raw = iop.tile([PP, ROWB], u8d, tag="raw")
                nc.sync.dma_start(out=raw[:], in_=xf[:, s, :])
                # last 8 bytes of each wire row are (row_step, row_min) f32
                aff = raw[:, W:ROWB].bitcast(f32)
                # decode: x = q*row_step + row_min, one tensor_scalar with
                # per-partition (per-row) f32 scalars.
                x = midp.tile([PP, CW], f16, tag="x")
                nc.vector.memset(x[:, W:CW], PAD_VAL)
                nc.vector.tensor_scalar(out=x[:, 0:W], in0=raw[:, 0:W],
                                        scalar1=aff[:, 0:1],
                                        scalar2=aff[:, 1:2],
                                        op0=mult, op1=add)

                u2 = midp.tile([PP, CW], f16, tag="u2")
                nc.vector.scalar_tensor_tensor(
                    out=u2[:, 0:CW - 1], in0=x[:, 1:CW], scalar=0.1,
                    in1=x[:, 0:CW - 1], op0=sub, op1=mx)
                u4 = midp.tile([PP, CW], f16, tag="u4")
                nc.vector.scalar_tensor_tensor(
                    out=u4[:, 0:CW - 3], in0=u2[:, 2:CW - 1], scalar=0.2,
                    in1=u2[:, 0:CW - 3], op0=sub, op1=mx)
                u8t = midp.tile([PP, CW], f16, tag="u8")
                nc.vector.scalar_tensor_tensor(
                    out=u8t[:, 0:CW - 7], in0=u4[:, 4:CW - 3], scalar=0.4,
                    in1=u4[:, 0:CW - 7], op0=sub, op1=mx)
                u16 = midp.tile([PP, CW], f16, tag="u16")
                nc.vector.scalar_tensor_tensor(
                    out=u16[:, 0:CW - 15], in0=u8t[:, 8:CW - 7], scalar=0.8,
                    in1=u8t[:, 0:CW - 15], op0=sub, op1=mx)

                d = midp.tile([PP, CW], f16, tag="d")
                nc.vector.scalar_tensor_tensor(
                    out=d[:, 0:W], in0=u16[:, 1:W + 1], scalar=0.1,
                    in1=x[:, 0:W], op0=sub, op1=sub)
                # t = clip(d, 0, 1); q6 = 63 - 63*t  (image = q6/63)
                # the DVE f16->u8 store rounds to nearest on HW (CoreSim
                # truncates), so no rounding bias is added here.
                t = midp.tile([PP, CW], f16, tag="t")
                nc.vector.tensor_scalar(
                    out=t[:, 0:W], in0=d[:, 0:W],
                    scalar1=0.0, scalar2=1.0, op0=mx, op1=mn)
                q6 = midp.tile([PP, W], u8d, tag="q6")
                nc.vector.tensor_scalar(
                    out=q6[:], in0=t[:, 0:W],
                    scalar1=-63.0, scalar2=63.0, op0=mult, op1=add)
                # pack 4x 6-bit -> 3 byte planes per row:
                #   b0 = q0 | (q1&3)<<6;  b1 = q1>>2 | (q2&15)<<4
                #   b2 = q2>>4 | q3<<2   (q3<<2 <= 252, no overflow)
                # (the walrus verifier rejects bitvec scalar_tensor_tensor
                # with immediates, so shifts go through tensor_scalar and
                # the combines through tensor_tensor)
                q64 = q6[:].rearrange("p (w four) -> p four w", four=4)
                zt = midp.tile([PP, 5 * Q], u8d, tag="zt")
                out = iop.tile([PP, OW], u8d, tag="out")
                z1, z2, z3 = zt[:, 0:Q], zt[:, Q:2 * Q], zt[:, 2 * Q:3 * Q]
                y1, y2 = zt[:, 3 * Q:4 * Q], zt[:, 4 * Q:5 * Q]
                nc.vector.tensor_scalar(out=z1, in0=q64[:, 1, :],
                                        scalar1=3, scalar2=6,
                                        op0=band, op1=shl)
                nc.vector.tensor_tensor(out=out[:, 0:Q], in0=q64[:, 0, :],
                                        in1=z1, op=bor)
                nc.vector.tensor_scalar(out=z2, in0=q64[:, 2, :],
                                        scalar1=15, scalar2=4,
                                        op0=band, op1=shl)
                nc.vector.tensor_scalar(out=y1, in0=q64[:, 1, :],
                                        scalar1=2, scalar2=None, op0=shr)
                nc.vector.tensor_tensor(out=out[:, Q:2 * Q], in0=y1,
                                        in1=z2, op=bor)
                nc.vector.tensor_scalar(out=z3, in0=q64[:, 3, :],
                                        scalar1=2, scalar2=None, op0=shl)
                nc.vector.tensor_scalar(out=y2, in0=q64[:, 2, :],
                                        scalar1=4, scalar2=None, op0=shr)
                nc.vector.tensor_tensor(out=out[:, 2 * Q:3 * Q], in0=y2,
                                        in1=z3, op=bor)
                nc.sync.dma_start(out=yf[:, s, :], in_=out[:])
    nc.compile()
    return nc


def _ensure_fast():
    """Build the Bass module and a cached jitted PJRT executable once.

    Mirrors the multi-core branch of bass2jax.run_bass_via_pjrt, but keeps
    the jax.jit wrapper (and with it the traced/lowered/compiled NEFF
    executable) alive across calls instead of rebuilding it per call.
    """
    if "fn" in _state:
        return
    import jax
    from jax.experimental.shard_map import shard_map
    from jax.sharding import Mesh, PartitionSpec

    bass2jax.install_neuronx_cc_hook()
    nc = _build_nc()

    partition_name = (nc.partition_id_tensor.name
                      if nc.partition_id_tensor else None)
    in_names = []
    out_names = []
    out_avals = []
    for alloc in nc.m.functions[0].allocations:
        if not isinstance(alloc, mybir.MemoryLocationSet):
            continue
        name = alloc.memorylocations[0].name
        if alloc.kind == "ExternalInput":
            if name != partition_name:
                in_names.append(name)
        elif alloc.kind == "ExternalOutput":
            shape = tuple(alloc.tensor_shape)
            dtype = mybir.dt.np(alloc.dtype)
            out_names.append(name)
            out_avals.append(jax.core.ShapedArray(shape, dtype))
    n_params = len(in_names)
    in_names = in_names + out_names  # donated output buffers come in as params
    if partition_name is not None:
        in_names.append(partition_name)

    def _body(*args):
        operands = list(args)
        if partition_name is not None:
            operands.append(bass2jax.partition_id_tensor())
        outs = bass2jax._bass_exec_p.bind(
            *operands,
            out_avals=tuple(out_avals),
            in_names=tuple(in_names),
            out_names=tuple(out_names),
            lowering_input_output_aliases=(),
            sim_require_finite=True,
            sim_require_nnan=True,
            nc=nc,
        )
        return tuple(outs)

    devices = jax.devices()[:N_CORES]
    mesh = Mesh(np.asarray(devices), ("core",))
    n_outs = len(out_names)
    fn = jax.jit(
        shard_map(_body, mesh=mesh,
                  in_specs=(PartitionSpec("core"),) * (n_params + n_outs),
                  out_specs=(PartitionSpec("core"),) * n_outs,
                  check_rep=False),
        donate_argnums=tuple(range(n_params, n_params + n_outs)),
        keep_unused=True,
    )
    _state["nc"] = nc
    _state["fn"] = fn
    # First call donates host zeros; afterwards we donate the previous
    # call's device-resident output arrays (already fetched to host).
    _state["donate"] = [np.zeros((B, C, HC, OW), np.uint8)
                        for _ in range(CHUNKS)]
    # Host-side scratch, reused across chunks/calls. packed is per-chunk
    # (its async upload outlives the pack loop iteration).
    _state["packed"] = [np.empty((B, C, HC, ROWB), np.uint8)
                        for _ in range(CHUNKS)]
    _state["fbuf"] = np.empty((B, C, HC, W), np.float32)
    _state["qful"] = np.empty((B, C, HC, W), np.uint8)
    _state["lut"] = (np.arange(64) / 63.0).astype(np.float32)
    from concurrent.futures import ThreadPoolExecutor
    _state["pool"] = ThreadPoolExecutor(1)


def _pack_chunk(xc, packed):
    """Quantize one [B,C,HC,W] f32 chunk to per-row 8-bit codes.

    q = round((x - rowmin) * 255/(rowmax-rowmin)); decode on device is
    x = q*rowstep + rowmin. Exact row bounds make any input wrap-proof.
    The two f32 affines live in the last 8 bytes of each 520B wire row.
    """
    fbuf = _state["fbuf"]
    mn = xc.min(axis=-1, keepdims=True)
    mx = xc.max(axis=-1, keepdims=True)
    diff = np.maximum(mx - mn, np.float32(1e-9))
    senc = np.float32(255.0) / diff
    np.multiply(xc, senc, out=fbuf)
    fbuf -= senc * mn - np.float32(0.5)        # +0.5: round via truncation
    np.copyto(packed[..., :W], fbuf, casting="unsafe")  # f32->u8 truncation
    affv = packed[..., W:].view(np.float32)
    affv[..., 0] = (diff * np.float32(1.0 / 255.0))[..., 0]
    affv[..., 1] = mn[..., 0]


def _decode_chunk(res_u8, view):
    """Unpack one [B,C,HC,OW] 6-bit plane chunk into f32 `view`."""
    qful, lut = _state["qful"], _state["lut"]
    b0 = res_u8[..., 0:Q]
    b1 = res_u8[..., Q:2 * Q]
    b2 = res_u8[..., 2 * Q:3 * Q]
    qful[..., 0::4] = b0 & 63
    qful[..., 1::4] = (b0 >> 6) | ((b1 & 15) << 2)
    qful[..., 2::4] = (b1 >> 4) | ((b2 & 3) << 4)
    qful[..., 3::4] = b2 >> 2
    np.take(lut, qful, out=view)


def _run_fast(heightfield: np.ndarray) -> np.ndarray:
    _ensure_fast()
    hf = np.asarray(heightfield, dtype=np.float32)
    assert hf.shape == (B, C, H, W), hf.shape
    fn = _state["fn"]
    donate = _state["donate"]
    pool = _state["pool"]
    result = np.empty((B, C, H, W), np.float32)

    def _fetch(i, o):
        res_u8 = np.asarray(o)                # blocks on chunk i only
        donate[i] = o                         # device buffer, donated next call
        _decode_chunk(res_u8, result[:, :, i * HC:(i + 1) * HC, :])

    # Dispatch all chunks asynchronously: uploads stream in order while
    # downloads of finished chunks flow back concurrently (duplex tunnel).
    # A single worker thread fetches + decodes finished chunks in order,
    # overlapping the remaining dispatches (numpy/jax release the GIL).
    futs = []
    for i in range(CHUNKS):
        packed = _state["packed"][i]
        _pack_chunk(hf[:, :, i * HC:(i + 1) * HC, :], packed)
        o = fn(packed, donate[i])[0]
        o.copy_to_host_async()
        futs.append(pool.submit(_fetch, i, o))
    for f in futs:
        f.result()
    return result


def kernel(heightfield: np.ndarray) -> np.ndarray:
    return _run_fast(heightfield)


# revision 21
# speedup vs baseline: 1.1130x; 1.0223x over previous
"""Trainium2 Bass kernel for sliding-window ridge/pooling op.

Reference computation (per [B,C,H,W]=[16,1,512,512] f32 input):
    padded = pad W axis right with 16 cols of -1000
    compare[w] = max_{r=1..16}( padded[w+r] - r/10 )
    image = 1 - clip(compare - x, 0, 1)

Algorithm: biased doubling. Define u_k[w] = max_{r=0..k-1}(x[w+r] - r/10).
  u_1 = x
  u_{2k}[w] = max(u_k[w], u_k[w+k] - k/10)      <- one scalar_tensor_tensor op
  compare[w] = u_16[w+1] - 0.1
So 4 STT steps + 1 final STT (d = (u16[w+1]-0.1) - x) + clip + quantize.

The on-chip kernel runs in ~10us; per-call cost is dominated by the axon
tunnel (~43MB/s combined capacity shared by both directions) plus
dispatch latency. So the whole design minimizes wire bytes and overlaps
everything:
  * input is quantized to 8-bit fixed point PER ROW on host (each
    512-pixel row gets its own f32 scale/offset from its exact min/max,
    making 8 bits as accurate as ~9.5 global bits and wrap-proof for any
    input); each 520B wire row = 512 u8 codes + 8 bytes of f32 affine
    (bitcast on device), decoded by one per-partition-scalar affine op,
  * output is quantized to 5 bits and packed 8 values -> 5 bytes on
    device (2.5MB down instead of 16MB), image = q/31 decoded on host,
  * the jitted PJRT executable is built ONCE and cached (the stock
    run_bass_kernel_spmd path re-traces and re-lowers on every call),
  * donated output buffers are the previous call's device-resident
    output arrays (no zeros upload per call),
  * the batch is cut into CHUNKS slices along H (window is along W, so
    no halo) and dispatched asynchronously with copy_to_host_async, so
    chunk i's download and host decode overlap chunk i+1's pack+upload.

Sharding: data-parallel over batch, 2 images per core on 8 cores.

Error budget (rel 2-norm, gate 2e-2): measured 1.25e-2 on the seed-0
input (per-row 8-bit input quant + f16 compute + 6-bit output quant;
~70% of output pixels are saturated at exactly 0 or 1 and carry no
quant noise).

Measured end-to-end: ~200-210ms/call (best-of-12), vs 838ms for the
staged baseline (f32 wire, per-call retrace, serial transfers). The
pipeline sits at the tunnel's shared-capacity floor for 7.06MB of wire
traffic (~152ms) plus ~10ms spin-up and ~40ms of tail latency.
"""

import numpy as np

try:
    from concourse import bacc, bass, bass2jax, mybir
    from concourse.tile import TileContext
except ImportError:  # fallback if site packages not on path
    import sys

    sys.path.insert(0, "/opt/trn_rl_repo")
    from concourse import bacc, bass, bass2jax, mybir
    from concourse.tile import TileContext

N_CORES = 8
B, C, H, W = 16, 1, 512, 512
PB = B // N_CORES            # batches per core = 2
P = 128                      # SBUF partitions
PAD_VAL = -1000.0
BUFW = W + 16                # 528: 512 data + 16 window pad (exact minimum)
ROWB = W + 8                 # 520 wire bytes/row: 512 u8 codes + 2 f32 affine
OW = (W * 5) // 8            # 320 output bytes/row: 5-bit packed, 5 planes
G = W // 8                   # 64 values per phase/plane

CHUNKS = 8                   # pipeline chunks along H
HC = H // CHUNKS             # rows per chunk
ROWS = PB * C * HC           # rows per core per chunk
PP = min(P, ROWS)            # partitions used per tile
SEGS = max(ROWS // P, 1)     # SBUF segments per core per chunk

_state = {}


def _build_nc():
    f16 = mybir.dt.float16
    f32 = mybir.dt.float32
    u8d = mybir.dt.uint8
    A = mybir.AluOpType
    sub, mx, mn, mult, add = A.subtract, A.max, A.min, A.mult, A.add
    band, shr, shl, bor = (A.bitwise_and, A.logical_shift_right,
                           A.logical_shift_left, A.bitwise_or)

    nc = bacc.Bacc("TRN2", target_bir_lowering=False, debug=False,
                   num_devices=N_CORES)
    x_dram = nc.dram_tensor("packed", [PB, C, HC, ROWB], u8d,
                            kind="ExternalInput").ap()
    y_dram = nc.dram_tensor("image", [PB, C, HC, OW], u8d,
                            kind="ExternalOutput").ap()
    xf = x_dram.flatten_outer_dims().rearrange("(s p) w -> p s w", p=PP)
    yf = y_dram.flatten_outer_dims().rearrange("(s p) w -> p s w", p=PP)

    CW = BUFW
    with TileContext(nc) as tc:
        with tc.tile_pool(name="io", bufs=SEGS) as iop, \
             tc.tile_pool(name="mid", bufs=SEGS) as midp:
            for s in range(SEGS):
                raw = iop.tile([PP, ROWB], u8d, tag="raw")
                nc.sync.dma_start(out=raw[:], in_=xf[:, s, :])
                # last 8 bytes of each wire row are (row_step, row_min) f32
                aff = raw[:, W:ROWB].bitcast(f32)
                # decode: x = q*row_step + row_min, one tensor_scalar with
                # per-partition (per-row) f32 scalars.
                x = midp.tile([PP, CW], f16, tag="x")
                nc.vector.memset(x[:, W:CW], PAD_VAL)
                nc.vector.tensor_scalar(out=x[:, 0:W], in0=raw[:, 0:W],
                                        scalar1=aff[:, 0:1],
                                        scalar2=aff[:, 1:2],
                                        op0=mult, op1=add)

                u2 = midp.tile([PP, CW], f16, tag="u2")
                nc.vector.scalar_tensor_tensor(
                    out=u2[:, 0:CW - 1], in0=x[:, 1:CW], scalar=0.1,
                    in1=x[:, 0:CW - 1], op0=sub, op1=mx)
                u4 = midp.tile([PP, CW], f16, tag="u4")
                nc.vector.scalar_tensor_tensor(
                    out=u4[:, 0:CW - 3], in0=u2[:, 2:CW - 1], scalar=0.2,
                    in1=u2[:, 0:CW - 3], op0=sub, op1=mx)
                u8t = midp.tile([PP, CW], f16, tag="u8")
                nc.vector.scalar_tensor_tensor(
                    out=u8t[:, 0:CW - 7], in0=u4[:, 4:CW - 3], scalar=0.4,
                    in1=u4[:, 0:CW - 7], op0=sub, op1=mx)
                u16 = midp.tile([PP, CW], f16, tag="u16")
                nc.vector.scalar_tensor_tensor(
                    out=u16[:, 0:CW - 15], in0=u8t[:, 8:CW - 7], scalar=0.8,
                    in1=u8t[:, 0:CW - 15], op0=sub, op1=mx)

                d = midp.tile([PP, CW], f16, tag="d")
                nc.vector.scalar_tensor_tensor(
                    out=d[:, 0:W], in0=u16[:, 1:W + 1], scalar=0.1,
                    in1=x[:, 0:W], op0=sub, op1=sub)
                # t = clip(d, 0, 1); q5 = 31 - 31*t  (image = q5/31)
                # the DVE f16->u8 store rounds to nearest on HW (CoreSim
                # truncates), so no rounding bias is added here.
                t = midp.tile([PP, CW], f16, tag="t")
                nc.vector.tensor_scalar(
                    out=t[:, 0:W], in0=d[:, 0:W],
                    scalar1=0.0, scalar2=1.0, op0=mx, op1=mn)
                q5 = midp.tile([PP, W], u8d, tag="q5")
                nc.vector.tensor_scalar(
                    out=q5[:], in0=t[:, 0:W],
                    scalar1=-31.0, scalar2=31.0, op0=mult, op1=add)
                # pack 8x 5-bit -> 5 byte planes per row (value i occupies
                # bits 5i..5i+4 of the 40-bit group; every shift operand is
                # masked first so nothing overflows u8):
                #   b0 = q0      | (q1&7)<<5
                #   b1 = q1>>3   | (q2)<<2   | (q3&1)<<7
                #   b2 = q3>>1   | (q4&15)<<4
                #   b3 = q4>>4   | (q5)<<1   | (q6&3)<<6
                #   b4 = q6>>2   | (q7)<<3
                # (the walrus verifier rejects bitvec scalar_tensor_tensor
                # with immediates, so shifts go through tensor_scalar and
                # the combines through tensor_tensor)
                q58 = q5[:].rearrange("p (w eight) -> p eight w", eight=8)
                zt = midp.tile([PP, 9 * G], u8d, tag="zt")
                out = iop.tile([PP, OW], u8d, tag="out")

                def Z(i):
                    return zt[:, i * G:(i + 1) * G]

                def plane(dst, lo_src, lo_shr, hi_src, hi_mask, hi_shl, zi):
                    nc.vector.tensor_scalar(out=Z(zi), in0=hi_src,
                                            scalar1=hi_mask, scalar2=hi_shl,
                                            op0=band, op1=shl)
                    nc.vector.tensor_scalar(out=Z(zi + 1), in0=lo_src,
                                            scalar1=lo_shr, scalar2=None,
                                            op0=shr)
                    nc.vector.tensor_tensor(out=dst, in0=Z(zi + 1),
                                            in1=Z(zi), op=bor)

                # b0: two sources (q0 needs no shift)
                nc.vector.tensor_scalar(out=Z(0), in0=q58[:, 1, :],
                                        scalar1=7, scalar2=5,
                                        op0=band, op1=shl)
                nc.vector.tensor_tensor(out=out[:, 0:G], in0=q58[:, 0, :],
                                        in1=Z(0), op=bor)
                # b1: three sources -> build (q2<<2 | q3&1<<7) then or q1>>3
                nc.vector.tensor_scalar(out=Z(1), in0=q58[:, 2, :],
                                        scalar1=31, scalar2=2,
                                        op0=band, op1=shl)
                nc.vector.tensor_scalar(out=Z(2), in0=q58[:, 3, :],
                                        scalar1=1, scalar2=7,
                                        op0=band, op1=shl)
                nc.vector.tensor_tensor(out=Z(3), in0=Z(1), in1=Z(2), op=bor)
                nc.vector.tensor_scalar(out=Z(4), in0=q58[:, 1, :],
                                        scalar1=3, scalar2=None, op0=shr)
                nc.vector.tensor_tensor(out=out[:, G:2 * G], in0=Z(4),
                                        in1=Z(3), op=bor)
                # b2
                plane(out[:, 2 * G:3 * G], q58[:, 3, :], 1,
                      q58[:, 4, :], 15, 4, 5)
                # b3: three sources
                nc.vector.tensor_scalar(out=Z(0), in0=q58[:, 5, :],
                                        scalar1=31, scalar2=1,
                                        op0=band, op1=shl)
                nc.vector.tensor_scalar(out=Z(1), in0=q58[:, 6, :],
                                        scalar1=3, scalar2=6,
                                        op0=band, op1=shl)
                nc.vector.tensor_tensor(out=Z(2), in0=Z(0), in1=Z(1), op=bor)
                nc.vector.tensor_scalar(out=Z(3), in0=q58[:, 4, :],
                                        scalar1=4, scalar2=None, op0=shr)
                nc.vector.tensor_tensor(out=out[:, 3 * G:4 * G], in0=Z(3),
                                        in1=Z(2), op=bor)
                # b4
                plane(out[:, 4 * G:5 * G], q58[:, 6, :], 2,
                      q58[:, 7, :], 31, 3, 7)
                nc.sync.dma_start(out=yf[:, s, :], in_=out[:])
    nc.compile()
    return nc


def _ensure_fast():
    """Build the Bass module and a cached jitted PJRT executable once.

    Mirrors the multi-core branch of bass2jax.run_bass_via_pjrt, but keeps
    the jax.jit wrapper (and with it the traced/lowered/compiled NEFF
    executable) alive across calls instead of rebuilding it per call.
    """
    if "fn" in _state:
        return
    import jax
    from jax.experimental.shard_map import shard_map
    from jax.sharding import Mesh, PartitionSpec

    bass2jax.install_neuronx_cc_hook()
    nc = _build_nc()

    partition_name = (nc.partition_id_tensor.name
                      if nc.partition_id_tensor else None)
    in_names = []
    out_names = []
    out_avals = []
    for alloc in nc.m.functions[0].allocations:
        if not isinstance(alloc, mybir.MemoryLocationSet):
            continue
        name = alloc.memorylocations[0].name
        if alloc.kind == "ExternalInput":
            if name != partition_name:
                in_names.append(name)
        elif alloc.kind == "ExternalOutput":
            shape = tuple(alloc.tensor_shape)
            dtype = mybir.dt.np(alloc.dtype)
            out_names.append(name)
            out_avals.append(jax.core.ShapedArray(shape, dtype))
    n_params = len(in_names)
    in_names = in_names + out_names  # donated output buffers come in as params
    if partition_name is not None:
        in_names.append(partition_name)

    def _body(*args):
        operands = list(args)
        if partition_name is not None:
            operands.append(bass2jax.partition_id_tensor())
        outs = bass2jax._bass_exec_p.bind(
            *operands,
            out_avals=tuple(out_avals),
            in_names=tuple(in_names),
            out_names=tuple(out_names),
            lowering_input_output_aliases=(),
            sim_require_finite=True,
            sim_require_nnan=True,
            nc=nc,
        )
        return tuple(outs)

    devices = jax.devices()[:N_CORES]
    mesh = Mesh(np.asarray(devices), ("core",))
    n_outs = len(out_names)
    fn = jax.jit(
        shard_map(_body, mesh=mesh,
                  in_specs=(PartitionSpec("core"),) * (n_params + n_outs),
                  out_specs=(PartitionSpec("core"),) * n_outs,
                  check_rep=False),
        donate_argnums=tuple(range(n_params, n_params + n_outs)),
        keep_unused=True,
    )
    _state["nc"] = nc
    _state["fn"] = fn
    # First call donates host zeros; afterwards we donate the previous
    # call's device-resident output arrays (already fetched to host).
    _state["donate"] = [np.zeros((B, C, HC, OW), np.uint8)
                        for _ in range(CHUNKS)]
    # Host-side scratch, reused across chunks/calls. packed is per-chunk
    # (its async upload outlives the pack loop iteration).
    _state["packed"] = [np.empty((B, C, HC, ROWB), np.uint8)
                        for _ in range(CHUNKS)]
    _state["fbuf"] = np.empty((B, C, HC, W), np.float32)
    _state["qful"] = np.empty((B, C, HC, W), np.uint8)
    _state["lut"] = (np.arange(32) / 31.0).astype(np.float32)
    from concurrent.futures import ThreadPoolExecutor
    _state["pool"] = ThreadPoolExecutor(1)


def _pack_chunk(xc, packed):
    """Quantize one [B,C,HC,W] f32 chunk to per-row 8-bit codes.

    q = round((x - rowmin) * 255/(rowmax-rowmin)); decode on device is
    x = q*rowstep + rowmin. Exact row bounds make any input wrap-proof.
    The two f32 affines live in the last 8 bytes of each 520B wire row.
    """
    fbuf = _state["fbuf"]
    mn = xc.min(axis=-1, keepdims=True)
    mx = xc.max(axis=-1, keepdims=True)
    diff = np.maximum(mx - mn, np.float32(1e-9))
    senc = np.float32(255.0) / diff
    np.multiply(xc, senc, out=fbuf)
    fbuf -= senc * mn - np.float32(0.5)        # +0.5: round via truncation
    np.copyto(packed[..., :W], fbuf, casting="unsafe")  # f32->u8 truncation
    affv = packed[..., W:].view(np.float32)
    affv[..., 0] = (diff * np.float32(1.0 / 255.0))[..., 0]
    affv[..., 1] = mn[..., 0]


def _decode_chunk(res_u8, view):
    """Unpack one [B,C,HC,OW] 5-bit plane chunk into f32 `view`."""
    qful, lut = _state["qful"], _state["lut"]
    b0 = res_u8[..., 0:G]
    b1 = res_u8[..., G:2 * G]
    b2 = res_u8[..., 2 * G:3 * G]
    b3 = res_u8[..., 3 * G:4 * G]
    b4 = res_u8[..., 4 * G:5 * G]
    qful[..., 0::8] = b0 & 31
    qful[..., 1::8] = (b0 >> 5) | ((b1 & 3) << 3)
    qful[..., 2::8] = (b1 >> 2) & 31
    qful[..., 3::8] = (b1 >> 7) | ((b2 & 15) << 1)
    qful[..., 4::8] = (b2 >> 4) | ((b3 & 1) << 4)
    qful[..., 5::8] = (b3 >> 1) & 31
    qful[..., 6::8] = (b3 >> 6) | ((b4 & 7) << 2)
    qful[..., 7::8] = b4 >> 3
    np.take(lut, qful, out=view)


def _run_fast(heightfield: np.ndarray) -> np.ndarray:
    _ensure_fast()
    hf = np.asarray(heightfield, dtype=np.float32)
    assert hf.shape == (B, C, H, W), hf.shape
    fn = _state["fn"]
    donate = _state["donate"]
    pool = _state["pool"]
    result = np.empty((B, C, H, W), np.float32)

    def _fetch(i, o):
        res_u8 = np.asarray(o)                # blocks on chunk i only
        donate[i] = o                         # device buffer, donated next call
        _decode_chunk(res_u8, result[:, :, i * HC:(i + 1) * HC, :])

    # Dispatch all chunks asynchronously: uploads stream in order while
    # downloads of finished chunks flow back concurrently (duplex tunnel).
    # A single worker thread fetches + decodes finished chunks in order,
    # overlapping the remaining dispatches (numpy/jax release the GIL).
    futs = []
    for i in range(CHUNKS):
        packed = _state["packed"][i]
        _pack_chunk(hf[:, :, i * HC:(i + 1) * HC, :], packed)
        o = fn(packed, donate[i])[0]
        o.copy_to_host_async()
        futs.append(pool.submit(_fetch, i, o))
    for f in futs:
        f.result()
    return result


def kernel(heightfield: np.ndarray) -> np.ndarray:
    return _run_fast(heightfield)
